# revision 2
# baseline (speedup 1.0000x reference)
"""Trainium2 Bass kernel for the CMIN video encoder (2x banded MHA + BiGRU).

Self-contained: builds one SPMD Bass program, shards batch across the
8 NeuronCores (8 batches each), runs via run_bass_kernel_spmd, and
reassembles the full [64, 256, 512] output on the host.

Layout strategy: activations feature-major ([feature, token]); projections
are lhsT=weightT matmuls. Banded attention computed per (b, h) on the two
128x128 diagonal score blocks plus two 3-wide corner blocks (the band never
leaves them); exp -> band-mask multiply -> PE column-sum -> reciprocal ->
single post-AV scale. q/k/v staged in bf16. The BiGRU runs both direction
chains interleaved; W_hh in fp8e4 (x64 prescale, un-scaled inside the gate
adds); h lives in fp16 inside a per-chunk SBUF y-tile that doubles as the
matmul moving operand and the y staging buffer (one DMA per 32 steps).
Sequence-length reversal and tail zeroing go through indirect-DMA row
gathers with host-built index tables, keeping the program SPMD-identical.
"""

import os
import numpy as np
import concourse.bass as bass
import concourse.bacc as bacc
import concourse.tile as tile
import concourse.mybir as mybir
from concourse.bass_utils import run_bass_kernel_spmd

B, T, D = 64, 256, 1024
H, DK = 8, D // 8
HID = 512
GH = HID >> 1          # 256
G3 = 3 * GH            # 768
ATTN_WIDTH = 3
NL = 2
NCORES = 8
BC = B // NCORES       # 8 batches per core
NTOK = BC * T          # 2048 token columns per core
SCALE = 1.0 / float(np.sqrt(DK))

F32 = mybir.dt.float32
F32R = mybir.dt.float32r
F16 = mybir.dt.float16
BF16 = mybir.dt.bfloat16
F8 = mybir.dt.float8e4
I32 = mybir.dt.int32
AF = mybir.ActivationFunctionType
ALU = mybir.AluOpType

KC = D // 128          # 8 contraction chunks for D
GC = G3 // 128         # 6 gate chunks
HC = GH // 128         # 2 hidden chunks
TT = NTOK // 512       # 4 token tiles of 512
TC = T // 128          # 2 chunks of the T axis
CH = 32                # recurrence steps per gx stream chunk
NCHUNK = T // CH

YROWS = 2 * BC * T + 1  # ystage rows incl. trailing zero row
ZROW = YROWS - 1

W8 = True               # W_hh in fp8e4 (prescaled); False -> fp16
WHH_SCALE = 64.0 if W8 else 1.0
WHH_INV = 1.0 / WHH_SCALE
CW = 262                # valid score columns: 2x128 diag + 2x3 corners


def _build(repeat: int = 1, phases: str = 'all'):
    nc = bacc.Bacc("TRN2", num_devices=NCORES)

    xT = nc.dram_tensor("xT", [D, NTOK], F32R, kind="ExternalInput")
    wq, wk, wv, wo = [], [], [], []
    for l in range(NL):
        wq.append(nc.dram_tensor(f"WqT{l}", [D, D], F32R, kind="ExternalInput"))
        wk.append(nc.dram_tensor(f"WkT{l}", [D, D], F32R, kind="ExternalInput"))
        wv.append(nc.dram_tensor(f"WvT{l}", [D, D], F32R, kind="ExternalInput"))
        wo.append(nc.dram_tensor(f"WoT{l}", [D, D], F32R, kind="ExternalInput"))
    b_attn = nc.dram_tensor("b_attn", [NL * 4, D], F32, kind="ExternalInput")
    wihf = nc.dram_tensor("WihFT", [D, G3], F32R, kind="ExternalInput")
    wihb = nc.dram_tensor("WihBT", [D, G3], F32R, kind="ExternalInput")
    whhf = nc.dram_tensor("WhhFT", [GH, G3], F8 if W8 else F16,
                          kind="ExternalInput")
    whhb = nc.dram_tensor("WhhBT", [GH, G3], F8 if W8 else F16,
                          kind="ExternalInput")
    band_d = nc.dram_tensor("band", [128, 264], BF16, kind="ExternalInput")
    ones_d = nc.dram_tensor("ones", [128, 128], BF16, kind="ExternalInput")
    iden_d = nc.dram_tensor("iden", [128, 128], F32, kind="ExternalInput")
    gxidx_d = nc.dram_tensor("gxidx", [128, NCHUNK * 2], I32, kind="ExternalInput")
    outidx_d = nc.dram_tensor("outidx", [128, BC * 4], I32, kind="ExternalInput")
    yout = nc.dram_tensor("yout", [BC, T, HID], F16, kind="ExternalOutput")

    with (
        nc.allow_low_precision(reason="bf16/fp16/fp8 staging is deliberate"),
        tile.TileContext(nc) as tc,
        tc.tile_pool(name="dram", bufs=1, space="DRAM") as dpool,
        tc.tile_pool(name="const", bufs=1) as cpool,
        tc.tile_pool(name="xs", bufs=1) as xpool,
        tc.tile_pool(name="stage", bufs=4) as spool,
        tc.tile_pool(name="psA", bufs=2, space="PSUM") as psA,
    ):
        qf_d = dpool.tile([H, 128, NTOK], BF16, name="qf_d")
        kf_d = dpool.tile([H, 128, NTOK], BF16, name="kf_d")
        vt_d = dpool.tile([BC, H, TC, 128, 128], BF16, name="vt_d")
        gxb_d = dpool.tile([BC * T, G3], F32, name="gxb_d")
        ystage = dpool.tile([YROWS, GH], F16, name="ystage")

        # ---- constants ---------------------------------------------------
        band_t = cpool.tile([128, 264], BF16, name="band_t")
        nc.sync.dma_start(band_t[:], band_d[:])
        ones_t = cpool.tile([128, 128], BF16, name="ones_t")
        nc.sync.dma_start(ones_t[:], ones_d[:])
        iden_t = cpool.tile([128, 128], F32, name="iden_t")
        nc.sync.dma_start(iden_t[:], iden_d[:])
        gxidx_t = cpool.tile([128, NCHUNK * 2], I32, name="gxidx_t")
        nc.sync.dma_start(gxidx_t[:], gxidx_d[:])
        outidx_t = cpool.tile([128, BC * 4], I32, name="outidx_t")
        nc.sync.dma_start(outidx_t[:], outidx_d[:])
        whh_t = cpool.tile([128, 2 * HC * G3], F8 if W8 else F16, name="whh_t")
        for dr, wd in enumerate((whhf, whhb)):
            for kc in range(HC):
                nc.sync.dma_start(
                    whh_t[:, (dr * HC + kc) * G3:(dr * HC + kc + 1) * G3],
                    wd[kc * 128:(kc + 1) * 128, :],
                )
        hzero = cpool.tile([128, 2 * HC * BC], F16, name="hzero")
        nc.vector.memset(hzero[:], 0.0)
        zrow_t = cpool.tile([128, HC], F16, name="zrow_t")
        nc.vector.memset(zrow_t[:], 0.0)
        nc.sync.dma_start(
            ystage[ZROW:ZROW + 1, :].rearrange("o (c p) -> p (o c)", p=128),
            zrow_t[:],
        )
        # fwd GRU input weights, resident for the streamed per-chunk pass
        wf_t = cpool.tile([128, KC * G3], F32R, name="wf_t")
        for kc in range(KC):
            nc.sync.dma_start(
                wf_t[:, kc * G3:(kc + 1) * G3], wihf[kc * 128:(kc + 1) * 128, :]
            )

        # ---- x resident (feature-major) ---------------------------------
        x_t = xpool.tile([128, KC * NTOK], F32R, name="x_t")
        for kc in range(KC):
            nc.sync.dma_start(
                x_t[:, kc * NTOK:(kc + 1) * NTOK], xT[kc * 128:(kc + 1) * 128, :]
            )

        def xsl(kc, c0=0, n=NTOK):
            return x_t[:, kc * NTOK + c0: kc * NTOK + c0 + n]

        def load_w_half(wpool, wdram, h0, hw):
            wt = wpool.tile([128, KC * 512], F32R, name="wt", tag="wt")
            for kc in range(KC):
                nc.sync.dma_start(
                    wt[:, kc * hw:(kc + 1) * hw],
                    wdram[kc * 128:(kc + 1) * 128, h0:h0 + hw],
                )
            return wt

        def attn_phase(wpool, aopool, bhpool, psB):
            ao_t = aopool.tile([128, H * NTOK], F32R, name="ao_t")
            for l in range(NL):
                # ============ PASS A: V, K, Q projections ============
                # V first (token-major), then K, then Q so the banded pass
                # can start on early heads while Q still computes.
                for half in range(2):
                    wt = load_w_half(wpool, wv[l], half * 512, 512)
                    for tc_i in range(NTOK // 128):
                        ps = psA.tile([128, 512], F32, name="psv", tag="psa")
                        for kc in range(KC):
                            nc.tensor.matmul(
                                ps[:],
                                xsl(kc, tc_i * 128, 128),
                                wt[:, kc * 512:(kc + 1) * 512],
                                start=(kc == 0),
                                stop=(kc == KC - 1),
                            )
                        st = spool.tile([128, 512], BF16, name="stv", tag="st")
                        nc.vector.tensor_copy(st[:], ps[:])
                        b_i, c_i = tc_i // TC, tc_i % TC
                        nc.sync.dma_start(
                            vt_d[b_i, half * 4:(half + 1) * 4, c_i, :, :]
                            .rearrange("h p d -> p (h d)"),
                            st[:],
                        )
                for which, (wdram, outd) in enumerate(((wk[l], kf_d), (wq[l], qf_d))):
                    for half in range(2):
                        wt = load_w_half(wpool, wdram, half * 512, 512)
                        for mcl in range(4):
                            mc = half * 4 + mcl
                            for tt in range(TT):
                                ps = psA.tile([128, 512], F32, name="psa", tag="psa")
                                for kc in range(KC):
                                    nc.tensor.matmul(
                                        ps[:],
                                        wt[:, kc * 512 + mcl * 128: kc * 512 + (mcl + 1) * 128],
                                        xsl(kc, tt * 512, 512),
                                        start=(kc == 0),
                                        stop=(kc == KC - 1),
                                    )
                                st = spool.tile([128, 512], BF16, name="st", tag="st")
                                nc.scalar.activation(st[:], ps[:], AF.Copy)
                                nc.sync.dma_start(
                                    outd[mc, :, tt * 512:(tt + 1) * 512], st[:]
                                )

                # ============ PASS B: banded attention per (b, h) ============
                for h in range(H):
                    for b0 in range(0, BC, 2):
                        dn = psB.tile([128, 512], F32, name="dn", tag="dn")
                        rr = bhpool.tile([128, 512], BF16, name="rr", tag="rr")
                        for bl in range(2):
                            b = b0 + bl
                            qbh = bhpool.tile([128, T], BF16, name="qbh", tag="qbh")
                            nc.sync.dma_start(qbh[:], qf_d[h, :, b * T:(b + 1) * T])
                            kbh = bhpool.tile([128, T], BF16, name="kbh", tag="kbh")
                            nc.sync.dma_start(kbh[:], kf_d[h, :, b * T:(b + 1) * T])
                            vbh = bhpool.tile([128, T], BF16, name="vbh", tag="vbh")
                            nc.sync.dma_start(
                                vbh[:].rearrange("p (c d) -> p c d", c=TC),
                                vt_d[b, h].rearrange("c p d -> p c d"),
                            )
                            ps = psB.tile([128, 264], F32, name="psst", tag="psst")
                            nc.tensor.matmul(ps[:, 0:128], kbh[:, 0:128],
                                             qbh[:, 0:128], start=True, stop=True)
                            nc.tensor.matmul(ps[:, 256:259], kbh[:, 0:128],
                                             qbh[:, 128:131], start=True, stop=True)
                            nc.tensor.matmul(ps[:, 128:256], kbh[:, 128:256],
                                             qbh[:, 128:256], start=True, stop=True)
                            nc.tensor.matmul(ps[:, 259:262], kbh[:, 128:256],
                                             qbh[:, 125:128], start=True, stop=True)
                            pe = bhpool.tile([128, 264], BF16, name="pe", tag="pe")
                            nc.scalar.activation(pe[:, 0:CW], ps[:, 0:CW],
                                                 AF.Exp, scale=SCALE)
                            pm = bhpool.tile([128, 264], BF16, name="pm", tag="pm")
                            nc.vector.tensor_mul(pm[:, 0:CW], pe[:, 0:CW],
                                                 band_t[:, 0:CW])
                            dsl = dn[:, bl * 256:(bl + 1) * 256]
                            nc.tensor.matmul(dsl[:, 0:128], ones_t[:],
                                             pm[:, 0:128], start=True, stop=False)
                            nc.tensor.matmul(dsl[:, 125:128], ones_t[:, 0:128],
                                             pm[:, 259:262], start=False, stop=True,
                                             skip_group_check=True)
                            nc.tensor.matmul(dsl[:, 128:256], ones_t[:],
                                             pm[:, 128:256], start=True, stop=False)
                            nc.tensor.matmul(dsl[:, 128:131], ones_t[:, 0:128],
                                             pm[:, 256:259], start=False, stop=True,
                                             skip_group_check=True)
                            av = psB.tile([128, 256], F32, name="av", tag="av")
                            nc.tensor.matmul(av[:, 0:128], vbh[:, 0:128],
                                             pm[:, 0:128], start=True, stop=False)
                            nc.tensor.matmul(av[:, 125:128], vbh[:, 128:256],
                                             pm[:, 259:262], start=False, stop=True,
                                             skip_group_check=True)
                            nc.tensor.matmul(av[:, 128:256], vbh[:, 128:256],
                                             pm[:, 128:256], start=True, stop=False)
                            nc.tensor.matmul(av[:, 128:131], vbh[:, 0:128],
                                             pm[:, 256:259], start=False, stop=True,
                                             skip_group_check=True)
                            if bl == 0:
                                avs = [av]
                            else:
                                avs.append(av)
                        nc.vector.reciprocal(rr[:], dn[:])
                        for bl in range(2):
                            b = b0 + bl
                            nc.vector.tensor_mul(
                                ao_t[:, h * NTOK + b * T: h * NTOK + (b + 1) * T],
                                avs[bl][:], rr[:, bl * 256:(bl + 1) * 256],
                            )

                # ============ PASS C: O projection + residual (in place) =====
                for half in range(2):
                    wt = load_w_half(wpool, wo[l], half * 512, 512)
                    for mcl in range(4):
                        mc = half * 4 + mcl
                        for tt in range(TT):
                            ps = psA.tile([128, 512], F32, name="pso", tag="psa")
                            for kc in range(KC):
                                nc.tensor.matmul(
                                    ps[:],
                                    wt[:, kc * 512 + mcl * 128: kc * 512 + (mcl + 1) * 128],
                                    ao_t[:, kc * NTOK + tt * 512: kc * NTOK + (tt + 1) * 512],
                                    start=(kc == 0),
                                    stop=(kc == KC - 1),
                                )
                            nc.vector.tensor_add(
                                xsl(mc, tt * 512, 512), ps[:], xsl(mc, tt * 512, 512)
                            )

            # ============ PASS D (bwd half): gx_bwd -> DRAM ============
            for half in range(2):
                wt = load_w_half(wpool, wihb, half * 384, 384)
                for mcl in range(3):
                    mc = half * 3 + mcl
                    for tt in range(TT):
                        ps = psA.tile([128, 512], F32, name="psg", tag="psa")
                        for kc in range(KC):
                            nc.tensor.matmul(
                                ps[:],
                                wt[:, kc * 384 + mcl * 128: kc * 384 + (mcl + 1) * 128],
                                xsl(kc, tt * 512, 512),
                                start=(kc == 0),
                                stop=(kc == KC - 1),
                            )
                        st = spool.tile([128, 512], F32, name="stg", tag="st")
                        nc.scalar.activation(st[:], ps[:], AF.Copy)
                        nc.sync.dma_start(
                            gxb_d[:, :]
                            .rearrange("(b t) g -> b t g", b=BC)[
                                tt * 2:(tt + 1) * 2, :, mc * 128:(mc + 1) * 128
                            ]
                            .rearrange("b t g -> g (b t)"),
                            st[:],
                        )

        def gru_phase(gxpool, recpool, psR):
            ytile_prev = None
            for ck in range(NCHUNK):
                gxs = gxpool.tile([128, CH * 96], F32, name="gxs", tag="gxs")
                # fwd gx: compute directly into SBUF for this time chunk
                for mc in range(GC):
                    ps = psR.tile([128, 256], F32, name="psf", tag="psf")
                    for kc in range(KC):
                        nc.tensor.matmul(
                            ps[:],
                            wf_t[:, kc * G3 + mc * 128: kc * G3 + (mc + 1) * 128],
                            x_t[:, kc * NTOK:(kc + 1) * NTOK]
                            .rearrange("p (b t) -> p b t", b=BC)[:, :, ck * CH:(ck + 1) * CH],
                            start=(kc == 0),
                            stop=(kc == KC - 1),
                        )
                    nc.vector.tensor_copy(
                        gxs[:, :]
                        .rearrange("p (j d c b) -> p j d c b", j=CH, d=2, c=GC)[
                            :, :, 0, mc, :
                        ].rearrange("p j b -> p b j"),
                        ps[:].rearrange("p (b j) -> p b j", b=BC),
                    )
                # bwd gx: indirect row gather in reverse_padded order + transpose
                for hf2 in range(2):
                    gb = gxpool.tile([128, G3], F32, name="gb", tag="gb", bufs=2)
                    nc.gpsimd.indirect_dma_start(
                        out=gb[:],
                        out_offset=None,
                        in_=gxb_d[:, :],
                        in_offset=bass.IndirectOffsetOnAxis(
                            ap=gxidx_t[:, ck * 2 + hf2: ck * 2 + hf2 + 1], axis=0
                        ),
                    )
                    for c in range(GC):
                        tp = psR.tile([128, 128], F32, name="tp", tag="tp")
                        nc.tensor.transpose(
                            tp[:], gb[:, c * 128:(c + 1) * 128], iden_t[:]
                        )
                        nc.vector.tensor_copy(
                            gxs[:, :]
                            .rearrange("p (j d c b) -> p j d c b", j=CH, d=2, c=GC)[
                                :, :, 1, c, hf2 * 4:(hf2 + 1) * 4
                            ]
                            .rearrange("p j b -> p b j"),
                            tp[:].rearrange("p (b j) -> p b j", b=4),
                        )
                # y tile: [128, (j, dr, c, b)] fp16; doubles as h storage
                ytile = recpool.tile([128, CH * 32], F16, name="ytile",
                                     tag="ytile", bufs=2)
                for jj in range(CH):
                    gsl = gxs[:, jj * 96:(jj + 1) * 96]
                    if jj == 0:
                        hsrc = hzero if ytile_prev is None else ytile_prev
                        hoff = 0 if ytile_prev is None else (CH - 1) * 32
                    else:
                        hsrc, hoff = ytile, (jj - 1) * 32
                    ps_g = psR.tile([128, 96], F32, name="ps_g", tag="ps_g")
                    for dr in range(2):
                        for c in range(GC):
                            for kc in range(HC):
                                nc.tensor.matmul(
                                    ps_g[:, dr * 48 + c * 8: dr * 48 + (c + 1) * 8],
                                    whh_t[:, (dr * HC + kc) * G3 + c * 128:
                                          (dr * HC + kc) * G3 + (c + 1) * 128],
                                    hsrc[:, hoff + dr * 16 + kc * 8:
                                         hoff + dr * 16 + (kc + 1) * 8],
                                    start=(kc == 0),
                                    stop=(kc == HC - 1),
                                )
                    for dr in range(2):
                        grz = recpool.tile([128, 32], F32, name="grz", tag=f"grz{dr}")
                        nc.vector.scalar_tensor_tensor(
                            grz[:], ps_g[:, dr * 48: dr * 48 + 32], WHH_INV,
                            gsl[:, dr * 48: dr * 48 + 32],
                            op0=ALU.mult, op1=ALU.add,
                        )
                        rz = recpool.tile([128, 32], F32, name="rz", tag=f"rz{dr}")
                        nc.scalar.activation(rz[:], grz[:], AF.Sigmoid)
                        t1 = recpool.tile([128, 16], F32, name="t1", tag=f"t1{dr}")
                        nc.vector.tensor_mul(
                            t1[:], rz[:, 0:16], ps_g[:, dr * 48 + 32: dr * 48 + 48]
                        )
                        t2 = recpool.tile([128, 16], F32, name="t2", tag=f"t2{dr}")
                        nc.gpsimd.scalar_tensor_tensor(
                            t2[:], t1[:], WHH_INV,
                            gsl[:, dr * 48 + 32: dr * 48 + 48],
                            op0=ALU.mult, op1=ALU.add,
                        )
                        n_t = recpool.tile([128, 16], F32, name="n_t", tag=f"n_t{dr}")
                        nc.scalar.activation(n_t[:], t2[:], AF.Tanh)
                        d_t = recpool.tile([128, 16], F32, name="d_t", tag=f"d_t{dr}")
                        nc.gpsimd.tensor_sub(
                            d_t[:], hsrc[:, hoff + dr * 16: hoff + (dr + 1) * 16],
                            n_t[:],
                        )
                        zd = recpool.tile([128, 16], F32, name="zd", tag=f"zd{dr}")
                        nc.vector.tensor_mul(zd[:], rz[:, 16:32], d_t[:])
                        nc.gpsimd.tensor_add(
                            ytile[:, jj * 32 + dr * 16: jj * 32 + (dr + 1) * 16],
                            n_t[:], zd[:],
                        )
                # stage this chunk's y to DRAM (one DMA per direction)
                for dr in range(2):
                    nc.sync.dma_start(
                        ystage[0:2 * BC * T, :]
                        .rearrange("(q t) (c p) -> q t c p", t=T, p=128)[
                            dr * BC:(dr + 1) * BC, ck * CH:(ck + 1) * CH, :, :
                        ],
                        ytile[:, :]
                        .rearrange("p (j d c b) -> d b j c p", j=CH, d=2, c=HC),
                    )
                ytile_prev = ytile

        for rep in range(repeat):
            if phases in ("all", "attn"):
                with (
                    tc.tile_pool(name="wt", bufs=2) as wpool,
                    tc.tile_pool(name="ao", bufs=1) as aopool,
                    tc.tile_pool(name="bh", bufs=3) as bhpool,
                    tc.tile_pool(name="psB", bufs=2, space="PSUM") as psB,
                ):
                    attn_phase(wpool, aopool, bhpool, psB)
            if phases in ("all", "gru"):
                with (
                    tc.tile_pool(name="gx", bufs=2) as gxpool,
                    tc.tile_pool(name="rec", bufs=3) as recpool,
                    tc.tile_pool(name="psR", bufs=2, space="PSUM") as psR,
                ):
                    gru_phase(gxpool, recpool, psR)

            # ============ PASS G: final assembly via row gather ============
            for b in range(BC):
                for sc in range(TC):
                    for dr in range(2):
                        col = b * 4 + dr * 2 + sc
                        yt = spool.tile([128, GH], F16, name="yt", tag="yt", bufs=4)
                        nc.gpsimd.indirect_dma_start(
                            out=yt[:],
                            out_offset=None,
                            in_=ystage[:, :],
                            in_offset=bass.IndirectOffsetOnAxis(
                                ap=outidx_t[:, col:col + 1], axis=0
                            ),
                        )
                        nc.sync.dma_start(
                            yout[b, sc * 128:(sc + 1) * 128, dr * GH:(dr + 1) * GH],
                            yt[:],
                        )

    nc.compile()
    return nc


_NC_CACHE = {}


def _get_nc(repeat: int = 1):
    if repeat not in _NC_CACHE:
        _NC_CACHE[repeat] = _build(repeat)
    return _NC_CACHE[repeat]


def _host_inputs(inputs, core):
    import ml_dtypes
    f8 = ml_dtypes.float8_e4m3

    bs = slice(core * BC, (core + 1) * BC)
    seg = np.asarray(inputs["seg_feats"][bs])
    seglen = np.asarray(inputs["seglen"][bs]).astype(np.int64)

    m = {
        "xT": np.ascontiguousarray(
            seg.transpose(2, 0, 1).reshape(D, NTOK), dtype=np.float32
        )
    }
    for l in range(NL):
        for nm_in, nm_out in (("Wq", "WqT"), ("Wk", "WkT"), ("Wv", "WvT"),
                              ("Wo", "WoT")):
            m[f"{nm_out}{l}"] = np.ascontiguousarray(
                np.asarray(inputs[nm_in][l]).T, dtype=np.float32
            )
    m["b_attn"] = np.stack(
        [np.asarray(inputs[f"b{w}"][l]) for l in range(NL) for w in "qkvo"]
    ).astype(np.float32)
    m["WihFT"] = np.ascontiguousarray(np.asarray(inputs["W_ih_f"]).T, np.float32)
    m["WihBT"] = np.ascontiguousarray(np.asarray(inputs["W_ih_b"]).T, np.float32)
    bhf = np.asarray(inputs["b_hh_f"]).astype(np.float32)
    bhb = np.asarray(inputs["b_hh_b"]).astype(np.float32)
    assert not np.any(bhf[2 * GH:]) and not np.any(bhb[2 * GH:]), \
        "nonzero b_hh_n not supported"
    # biases are all zero in this model; gate adds skip them entirely
    assert not np.any(np.asarray(inputs["b_ih_f"])) \
        and not np.any(np.asarray(inputs["b_ih_b"])) \
        and not np.any(bhf) and not np.any(bhb), "nonzero GRU biases unsupported"
    for l in range(NL):
        for w in "qkvo":
            assert not np.any(np.asarray(inputs[f"b{w}"][l])), \
                "nonzero attention biases unsupported"
    wtype = f8 if W8 else np.float16
    m["WhhFT"] = np.ascontiguousarray(
        np.asarray(inputs["W_hh_f"]).T * WHH_SCALE).astype(wtype)
    m["WhhBT"] = np.ascontiguousarray(
        np.asarray(inputs["W_hh_b"]).T * WHH_SCALE).astype(wtype)

    # band mask: two 128x128 diagonal blocks + two 3-wide corner blocks
    band = np.zeros((128, 264), np.float32)
    p = np.arange(128)
    for c in range(2):
        band[:, c * 128:(c + 1) * 128] = (
            np.abs(p[:, None] - p[None, :]) <= ATTN_WIDTH
        )
    for j in range(3):
        for pp in range(125, 128):           # corner A: k=pp, q=128+j
            if abs(pp - 128 - j) <= ATTN_WIDTH:
                band[pp, 256 + j] = 1.0
        for pp in range(0, 3):               # corner B: k=128+pp, q=125+j
            if abs(128 + pp - 125 - j) <= ATTN_WIDTH:
                band[pp, 259 + j] = 1.0
    m["band"] = band.astype(ml_dtypes.bfloat16)
    m["ones"] = np.ones((128, 128), ml_dtypes.bfloat16)
    m["iden"] = np.eye(128, dtype=np.float32)

    gxidx = np.zeros((128, NCHUNK * 2), np.int32)
    for ck in range(NCHUNK):
        for hf2 in range(2):
            col = ck * 2 + hf2
            for bl in range(4):
                b = hf2 * 4 + bl
                L = int(seglen[b])
                for jl in range(CH):
                    j = ck * CH + jl
                    src_t = min(max(L - 1 - j, 0), T - 1)
                    gxidx[bl * CH + jl, col] = b * T + src_t
    m["gxidx"] = gxidx

    outidx = np.zeros((128, BC * 4), np.int32)
    for b in range(BC):
        L = int(seglen[b])
        for dr in range(2):
            for sc in range(TC):
                col = b * 4 + dr * 2 + sc
                for p_i in range(128):
                    s = sc * 128 + p_i
                    if s < L:
                        jrow = s if dr == 0 else L - 1 - s
                        outidx[p_i, col] = (dr * BC + b) * T + jrow
                    else:
                        outidx[p_i, col] = ZROW
    m["outidx"] = outidx
    return m


def kernel(**inputs) -> np.ndarray:
    repeat = int(os.environ.get("KERNEL_REPEAT", "1"))
    nc = _get_nc(repeat)
    in_maps = [_host_inputs(inputs, c) for c in range(NCORES)]
    res = run_bass_kernel_spmd(nc, in_maps, core_ids=list(range(NCORES)))
    out = np.concatenate([res.results[c]["yout"] for c in range(NCORES)], axis=0)
    return np.ascontiguousarray(out, dtype=np.float32)


# revision 20
# speedup vs baseline: 2.7622x; 2.7622x over previous
"""Trainium2 Bass kernel for the CMIN video encoder (2x banded MHA + BiGRU).

Self-contained: builds one SPMD Bass program, shards batch across the
8 NeuronCores (8 batches each), runs via run_bass_kernel_spmd, and
reassembles the full [64, 256, 512] output on the host.

Layout strategy: activations feature-major ([feature, token]); projections
are lhsT=weightT matmuls. Banded attention computed per (b, h) on the two
128x128 diagonal score blocks plus two 3-wide corner blocks (the band never
leaves them); q/k/v staged in bf16 and loaded once per head. The BiGRU runs
both direction chains interleaved; W_hh in fp8e4 (x64 prescale, un-scaled
inside the gate adds); h lives in fp32 in a per-chunk SBUF y-tile (with an
fp16 shadow as the matmul moving operand). Every 32 steps the y-tile is
PE-transposed to token-major and indirect-DMA-scattered straight into the
output tensor - sequence reversal, placement and tail masking all encoded
in host-built row-index tables, keeping the program SPMD-identical.
"""

import os
import numpy as np
import concourse.bass as bass
import concourse.bacc as bacc
import concourse.tile as tile
import concourse.mybir as mybir
from concourse.bass_utils import run_bass_kernel_spmd

B, T, D = 64, 256, 1024
H, DK = 8, D // 8
HID = 512
GH = HID >> 1          # 256
G3 = 3 * GH            # 768
ATTN_WIDTH = 3
NL = 2
NCORES = 8
BC = B // NCORES       # 8 batches per core
NTOK = BC * T          # 2048 token columns per core
SCALE = 1.0 / float(np.sqrt(DK))

F32 = mybir.dt.float32
F32R = mybir.dt.float32r
F16 = mybir.dt.float16
BF16 = mybir.dt.bfloat16
F8 = mybir.dt.float8e4
I32 = mybir.dt.int32
AF = mybir.ActivationFunctionType
ALU = mybir.AluOpType

KC = D // 128          # 8 contraction chunks for D
GC = G3 // 128         # 6 gate chunks
HC = GH // 128         # 2 hidden chunks
TT = NTOK // 512       # 4 token tiles of 512
TC = T // 128          # 2 chunks of the T axis
CH = 32                # recurrence steps per gx stream chunk
NCHUNK = T // CH

YR = BC * T            # valid yout rows; row YR is the trash row
W8 = True              # W_hh in fp8e4 (prescaled); False -> fp16
WHH_SCALE = 64.0 if W8 else 1.0
WHH_INV = 1.0 / WHH_SCALE
CW = 262               # valid score columns: 2x128 diag + 2x3 corners


def _build(repeat: int = 1, phases: str = 'all'):
    nc = bacc.Bacc("TRN2", num_devices=NCORES)

    xT = nc.dram_tensor("xT", [D, NTOK], F32R, kind="ExternalInput")
    wq, wk, wv, wo = [], [], [], []
    for l in range(NL):
        wq.append(nc.dram_tensor(f"WqT{l}", [D, D], F32R, kind="ExternalInput"))
        wk.append(nc.dram_tensor(f"WkT{l}", [D, D], F32R, kind="ExternalInput"))
        wv.append(nc.dram_tensor(f"WvT{l}", [D, D], F32R, kind="ExternalInput"))
        wo.append(nc.dram_tensor(f"WoT{l}", [D, D], BF16, kind="ExternalInput"))
    wihf = nc.dram_tensor("WihFT", [D, G3], F32R, kind="ExternalInput")
    wihb = nc.dram_tensor("WihBT", [D, G3], F32R, kind="ExternalInput")
    whhf = nc.dram_tensor("WhhFT", [GH, G3], F8 if W8 else F16,
                          kind="ExternalInput")
    whhb = nc.dram_tensor("WhhBT", [GH, G3], F8 if W8 else F16,
                          kind="ExternalInput")
    band_d = nc.dram_tensor("band", [128, 264], BF16, kind="ExternalInput")
    ones_d = nc.dram_tensor("ones", [128, 128], BF16, kind="ExternalInput")
    iden_d = nc.dram_tensor("iden", [128, 128], F32, kind="ExternalInput")
    gxidx_d = nc.dram_tensor("gxidx", [128, NCHUNK * 2], I32, kind="ExternalInput")
    sidx_d = nc.dram_tensor("sidx", [128, NCHUNK * 4], I32, kind="ExternalInput")
    yout = nc.dram_tensor("yout", [YR + 1, HID], F16, kind="ExternalOutput")

    with (
        nc.allow_low_precision(reason="bf16/fp16/fp8 staging is deliberate"),
        tile.TileContext(nc) as tc,
        tc.tile_pool(name="dram", bufs=1, space="DRAM") as dpool,
        tc.tile_pool(name="const", bufs=1) as cpool,
        tc.tile_pool(name="xs", bufs=1) as xpool,
        tc.tile_pool(name="stage", bufs=4) as spool,
        tc.tile_pool(name="psA", bufs=2, space="PSUM") as psA,
    ):
        qf_d = dpool.tile([H, 128, NTOK], BF16, name="qf_d")
        kf_d = dpool.tile([H, 128, NTOK], BF16, name="kf_d")
        vt_d = dpool.tile([BC, H, TC, 128, 128], BF16, name="vt_d")
        gxb_d = dpool.tile([BC * T, G3], F32, name="gxb_d")

        # ---- constants ---------------------------------------------------
        band_t = cpool.tile([128, 264], BF16, name="band_t")
        nc.sync.dma_start(band_t[:], band_d[:])
        ones_t = cpool.tile([128, 128], BF16, name="ones_t")
        nc.sync.dma_start(ones_t[:], ones_d[:])
        iden_t = cpool.tile([128, 128], F32, name="iden_t")
        nc.sync.dma_start(iden_t[:], iden_d[:])
        gxidx_t = cpool.tile([128, NCHUNK * 2], I32, name="gxidx_t")
        nc.sync.dma_start(gxidx_t[:], gxidx_d[:])
        sidx_t = cpool.tile([128, NCHUNK * 4], I32, name="sidx_t")
        nc.sync.dma_start(sidx_t[:], sidx_d[:])
        whh_t = cpool.tile([128, 2 * HC * G3], F8 if W8 else F16, name="whh_t")
        for dr, wd in enumerate((whhf, whhb)):
            nc.sync.dma_start(
                whh_t[:, dr * HC * G3:(dr + 1) * HC * G3]
                .rearrange("p (c g) -> p c g", c=HC),
                wd[:, :].rearrange("(c p) g -> p c g", p=128),
            )
        hzero = cpool.tile([128, 2 * HC * BC], F16, name="hzero")
        nc.vector.memset(hzero[:], 0.0)
        hzero32 = cpool.tile([128, 2 * HC * BC], F32, name="hzero32")
        nc.vector.memset(hzero32[:], 0.0)

        # ---- x resident (feature-major) ---------------------------------
        x_t = xpool.tile([128, KC * NTOK], F32R, name="x_t")
        nc.sync.dma_start(
            x_t[:].rearrange("p (c n) -> p c n", c=KC),
            xT[:, :].rearrange("(c p) n -> p c n", p=128),
        )

        def xsl(kc, c0=0, n=NTOK):
            return x_t[:, kc * NTOK + c0: kc * NTOK + c0 + n]

        def load_w_half(wpool, wdram, h0, hw, dt=F32R):
            wt = wpool.tile([128, KC * 512], dt, name="wt", tag="wt")
            nc.sync.dma_start(
                wt[:, 0:KC * hw].rearrange("p (c w) -> p c w", c=KC),
                wdram[:, h0:h0 + hw].rearrange("(c p) w -> p c w", p=128),
            )
            return wt

        def attn_phase(wpool, aopool, bhpool, psB):
            ao_t = aopool.tile([128, H * NTOK], BF16, name="ao_t")
            for l in range(NL):
                # ============ PASS A: V, K, Q projections ============
                for half in range(2):
                    wt = load_w_half(wpool, wv[l], half * 512, 512)
                    for tc_i in range(NTOK // 128):
                        ps = psA.tile([128, 512], F32, name="psv", tag="psa")
                        for kc in range(KC):
                            nc.tensor.matmul(
                                ps[:],
                                xsl(kc, tc_i * 128, 128),
                                wt[:, kc * 512:(kc + 1) * 512],
                                start=(kc == 0),
                                stop=(kc == KC - 1),
                            )
                        st = spool.tile([128, 512], BF16, name="stv", tag="st")
                        nc.vector.tensor_copy(st[:], ps[:])
                        b_i, c_i = tc_i // TC, tc_i % TC
                        nc.sync.dma_start(
                            vt_d[b_i, half * 4:(half + 1) * 4, c_i, :, :]
                            .rearrange("h p d -> p h d"),
                            st[:].rearrange("p (h d) -> p h d", h=4),
                        )
                for which, (wdram, outd) in enumerate(((wk[l], kf_d), (wq[l], qf_d))):
                    for half in range(2):
                        wt = load_w_half(wpool, wdram, half * 512, 512)
                        for mcl in range(4):
                            mc = half * 4 + mcl
                            for tt in range(TT):
                                ps = psA.tile([128, 512], F32, name="psa", tag="psa")
                                for kc in range(KC):
                                    nc.tensor.matmul(
                                        ps[:],
                                        wt[:, kc * 512 + mcl * 128: kc * 512 + (mcl + 1) * 128],
                                        xsl(kc, tt * 512, 512),
                                        start=(kc == 0),
                                        stop=(kc == KC - 1),
                                    )
                                st = spool.tile([128, 512], BF16, name="st", tag="st")
                                nc.scalar.activation(st[:], ps[:], AF.Copy)
                                nc.sync.dma_start(
                                    outd[mc, :, tt * 512:(tt + 1) * 512], st[:]
                                )

                # ============ PASS B: banded attention per (b, h) ============
                for h in range(H):
                    qh = bhpool.tile([128, NTOK], BF16, name="qh", tag="qh")
                    nc.sync.dma_start(qh[:], qf_d[h])
                    kh = bhpool.tile([128, NTOK], BF16, name="kh", tag="kh")
                    nc.sync.dma_start(kh[:], kf_d[h])
                    vh = bhpool.tile([128, NTOK], BF16, name="vh", tag="vh")
                    for c in range(TC):
                        nc.sync.dma_start(
                            vh[:].rearrange("p (b c d) -> p b c d", b=BC, c=TC)[
                                :, :, c, :
                            ],
                            vt_d[:, h, c].rearrange("b p d -> p b d"),
                        )
                    for b0 in range(0, BC, 2):
                        dn = psB.tile([128, 512], F32, name="dn", tag="dn")
                        rr = bhpool.tile([128, 512], BF16, name="rr", tag="rr")
                        avs = []
                        for bl in range(2):
                            b = b0 + bl
                            qb = qh[:, b * T:(b + 1) * T]
                            kb = kh[:, b * T:(b + 1) * T]
                            vb = vh[:, b * T:(b + 1) * T]
                            ps = psB.tile([128, 264], F32, name="psst", tag="psst")
                            nc.tensor.matmul(ps[:, 0:128], kb[:, 0:128],
                                             qb[:, 0:128], start=True, stop=True)
                            nc.tensor.matmul(ps[:, 256:259], kb[:, 0:128],
                                             qb[:, 128:131], start=True, stop=True)
                            nc.tensor.matmul(ps[:, 128:256], kb[:, 128:256],
                                             qb[:, 128:256], start=True, stop=True)
                            nc.tensor.matmul(ps[:, 259:262], kb[:, 128:256],
                                             qb[:, 125:128], start=True, stop=True)
                            pe = bhpool.tile([128, 264], BF16, name="pe", tag="pe")
                            nc.scalar.activation(pe[:, 0:CW], ps[:, 0:CW],
                                                 AF.Exp, scale=SCALE)
                            pm = bhpool.tile([128, 264], BF16, name="pm", tag="pm")
                            nc.vector.tensor_mul(pm[:, 0:CW], pe[:, 0:CW],
                                                 band_t[:, 0:CW])
                            dsl = dn[:, bl * 256:(bl + 1) * 256]
                            nc.tensor.matmul(dsl[:, 0:128], ones_t[:],
                                             pm[:, 0:128], start=True, stop=False)
                            nc.tensor.matmul(dsl[:, 125:128], ones_t[:, 0:128],
                                             pm[:, 259:262], start=False, stop=True,
                                             skip_group_check=True)
                            nc.tensor.matmul(dsl[:, 128:256], ones_t[:],
                                             pm[:, 128:256], start=True, stop=False)
                            nc.tensor.matmul(dsl[:, 128:131], ones_t[:, 0:128],
                                             pm[:, 256:259], start=False, stop=True,
                                             skip_group_check=True)
                            av = psB.tile([128, 256], F32, name="av", tag="av")
                            nc.tensor.matmul(av[:, 0:128], vb[:, 0:128],
                                             pm[:, 0:128], start=True, stop=False)
                            nc.tensor.matmul(av[:, 125:128], vb[:, 128:256],
                                             pm[:, 259:262], start=False, stop=True,
                                             skip_group_check=True)
                            nc.tensor.matmul(av[:, 128:256], vb[:, 128:256],
                                             pm[:, 128:256], start=True, stop=False)
                            nc.tensor.matmul(av[:, 128:131], vb[:, 0:128],
                                             pm[:, 256:259], start=False, stop=True,
                                             skip_group_check=True)
                            avs.append(av)
                        nc.vector.reciprocal(rr[:], dn[:])
                        for bl in range(2):
                            b = b0 + bl
                            nc.vector.tensor_mul(
                                ao_t[:, h * NTOK + b * T: h * NTOK + (b + 1) * T],
                                avs[bl][:], rr[:, bl * 256:(bl + 1) * 256],
                            )

                # ============ PASS C: O projection + residual (in place) =====
                for half in range(2):
                    wt = load_w_half(wpool, wo[l], half * 512, 512, dt=BF16)
                    for mcl in range(4):
                        mc = half * 4 + mcl
                        for tt in range(TT):
                            ps = psA.tile([128, 512], F32, name="pso", tag="psa")
                            for kc in range(KC):
                                nc.tensor.matmul(
                                    ps[:],
                                    wt[:, kc * 512 + mcl * 128: kc * 512 + (mcl + 1) * 128],
                                    ao_t[:, kc * NTOK + tt * 512: kc * NTOK + (tt + 1) * 512],
                                    start=(kc == 0),
                                    stop=(kc == KC - 1),
                                )
                            nc.vector.tensor_add(
                                xsl(mc, tt * 512, 512), ps[:], xsl(mc, tt * 512, 512)
                            )

            # ============ PASS D (bwd half): gx_bwd -> DRAM ============
            for half in range(2):
                wt = load_w_half(wpool, wihb, half * 384, 384)
                for mcl in range(3):
                    mc = half * 3 + mcl
                    for tt in range(TT):
                        ps = psA.tile([128, 512], F32, name="psg", tag="psa")
                        for kc in range(KC):
                            nc.tensor.matmul(
                                ps[:],
                                wt[:, kc * 384 + mcl * 128: kc * 384 + (mcl + 1) * 128],
                                xsl(kc, tt * 512, 512),
                                start=(kc == 0),
                                stop=(kc == KC - 1),
                            )
                        st = spool.tile([128, 512], F32, name="stg", tag="st")
                        nc.scalar.activation(st[:], ps[:], AF.Copy)
                        nc.sync.dma_start(
                            gxb_d[:, :]
                            .rearrange("(b t) g -> b t g", b=BC)[
                                tt * 2:(tt + 1) * 2, :, mc * 128:(mc + 1) * 128
                            ]
                            .rearrange("b t g -> g (b t)"),
                            st[:],
                        )

        def gru_phase(gxpool, recpool, psR):
            wf_t = gxpool.tile([128, KC * G3], F32R, name="wf_t", tag="wf", bufs=1)
            nc.sync.dma_start(
                wf_t[:].rearrange("p (c g) -> p c g", c=KC),
                wihf[:, :].rearrange("(c p) g -> p c g", p=128),
            )
            yprev, h16prev = None, None
            for ck in range(NCHUNK):
                gxs = gxpool.tile([128, CH * 96], F32, name="gxs", tag="gxs")
                # fwd gx: compute directly into SBUF for this time chunk
                for mc in range(GC):
                    ps = psR.tile([128, 256], F32, name="psf", tag="psf")
                    for kc in range(KC):
                        nc.tensor.matmul(
                            ps[:],
                            wf_t[:, kc * G3 + mc * 128: kc * G3 + (mc + 1) * 128],
                            x_t[:, kc * NTOK:(kc + 1) * NTOK]
                            .rearrange("p (b t) -> p b t", b=BC)[:, :, ck * CH:(ck + 1) * CH],
                            start=(kc == 0),
                            stop=(kc == KC - 1),
                        )
                    nc.vector.tensor_copy(
                        gxs[:, :]
                        .rearrange("p (j d c b) -> p j d c b", j=CH, d=2, c=GC)[
                            :, :, 0, mc, :
                        ].rearrange("p j b -> p b j"),
                        ps[:].rearrange("p (b j) -> p b j", b=BC),
                    )
                # bwd gx: indirect row gather in reverse_padded order + transpose
                for hf2 in range(2):
                    gb = gxpool.tile([128, G3], F32, name="gb", tag="gb", bufs=2)
                    nc.gpsimd.indirect_dma_start(
                        out=gb[:],
                        out_offset=None,
                        in_=gxb_d[:, :],
                        in_offset=bass.IndirectOffsetOnAxis(
                            ap=gxidx_t[:, ck * 2 + hf2: ck * 2 + hf2 + 1], axis=0
                        ),
                    )
                    for c in range(GC):
                        tp = psR.tile([128, 128], F32, name="tp", tag="tp")
                        nc.tensor.transpose(
                            tp[:], gb[:, c * 128:(c + 1) * 128], iden_t[:]
                        )
                        nc.vector.tensor_copy(
                            gxs[:, :]
                            .rearrange("p (j d c b) -> p j d c b", j=CH, d=2, c=GC)[
                                :, :, 1, c, hf2 * 4:(hf2 + 1) * 4
                            ]
                            .rearrange("p j b -> p b j"),
                            tp[:].rearrange("p (b j) -> p b j", b=4),
                        )
                # y tile: [128, (j, dr, c, b)] fp32 h/y storage + fp16 shadow
                ytile = recpool.tile([128, CH * 32], F32, name="ytile",
                                     tag="ytile", bufs=2)
                h16t = recpool.tile([128, CH * 32], F16, name="h16t",
                                    tag="h16t", bufs=2)
                for jj in range(CH):
                    gsl = gxs[:, jj * 96:(jj + 1) * 96]
                    if jj == 0:
                        hs32, hs16 = (hzero32, hzero) if yprev is None \
                            else (yprev, h16prev)
                        hoff = 0 if yprev is None else (CH - 1) * 32
                    else:
                        hs32, hs16, hoff = ytile, h16t, (jj - 1) * 32
                    ps_g = psR.tile([128, 96], F32, name="ps_g", tag="ps_g")
                    for dr in range(2):
                        for c in range(GC):
                            for kc in range(HC):
                                nc.tensor.matmul(
                                    ps_g[:, dr * 48 + c * 8: dr * 48 + (c + 1) * 8],
                                    whh_t[:, (dr * HC + kc) * G3 + c * 128:
                                          (dr * HC + kc) * G3 + (c + 1) * 128],
                                    hs16[:, hoff + dr * 16 + kc * 8:
                                         hoff + dr * 16 + (kc + 1) * 8],
                                    start=(kc == 0),
                                    stop=(kc == HC - 1),
                                )
                    for dr in range(2):
                        grz = recpool.tile([128, 32], F32, name="grz", tag=f"grz{dr}")
                        nc.vector.scalar_tensor_tensor(
                            grz[:], ps_g[:, dr * 48: dr * 48 + 32], WHH_INV,
                            gsl[:, dr * 48: dr * 48 + 32],
                            op0=ALU.mult, op1=ALU.add,
                        )
                        rz = recpool.tile([128, 32], F32, name="rz", tag=f"rz{dr}")
                        nc.scalar.activation(rz[:], grz[:], AF.Sigmoid)
                        t1 = recpool.tile([128, 16], F32, name="t1", tag=f"t1{dr}")
                        nc.vector.tensor_mul(
                            t1[:], rz[:, 0:16], ps_g[:, dr * 48 + 32: dr * 48 + 48]
                        )
                        t2 = recpool.tile([128, 16], F32, name="t2", tag=f"t2{dr}")
                        nc.vector.scalar_tensor_tensor(
                            t2[:], t1[:], WHH_INV,
                            gsl[:, dr * 48 + 32: dr * 48 + 48],
                            op0=ALU.mult, op1=ALU.add,
                        )
                        n_t = recpool.tile([128, 16], F32, name="n_t", tag=f"n_t{dr}")
                        nc.scalar.activation(n_t[:], t2[:], AF.Tanh)
                        d_t = recpool.tile([128, 16], F32, name="d_t", tag=f"d_t{dr}")
                        nc.gpsimd.tensor_sub(
                            d_t[:], hs32[:, hoff + dr * 16: hoff + (dr + 1) * 16],
                            n_t[:],
                        )
                        zd = recpool.tile([128, 16], F32, name="zd", tag=f"zd{dr}")
                        nc.gpsimd.tensor_mul(zd[:], rz[:, 16:32], d_t[:])
                        ysl = ytile[:, jj * 32 + dr * 16: jj * 32 + (dr + 1) * 16]
                        nc.gpsimd.tensor_add(ysl, n_t[:], zd[:])
                        nc.vector.tensor_copy(
                            h16t[:, jj * 32 + dr * 16: jj * 32 + (dr + 1) * 16], ysl
                        )
                # transpose to token-major and scatter into yout
                for dr in range(2):
                    for jh in range(2):
                        yrp = recpool.tile([128, 256], F32, name="yrp",
                                           tag="yrp", bufs=2)
                        for c in range(HC):
                            nc.vector.tensor_copy(
                                yrp[:, c * 128:(c + 1) * 128]
                                .rearrange("p (j b) -> p j b", j=16),
                                ytile[:, :]
                                .rearrange("p (j d c b) -> p j d c b",
                                           j=CH, d=2, c=HC)[
                                    :, jh * 16:(jh + 1) * 16, dr, c, :
                                ],
                            )
                        tp = psR.tile([128, 256], F32, name="tps", tag="tp")
                        for c in range(HC):
                            nc.tensor.transpose(
                                tp[:, c * 128:(c + 1) * 128],
                                yrp[:, c * 128:(c + 1) * 128],
                                iden_t[:],
                            )
                        yst = recpool.tile([128, 256], F16, name="yst",
                                           tag="yst", bufs=3)
                        nc.vector.tensor_copy(yst[:], tp[:])
                        col = ck * 4 + dr * 2 + jh
                        # sidx holds 2*row so coef=GH lands on row*HID; the
                        # direction's column offset rides in element_offset.
                        nc.gpsimd.indirect_dma_start(
                            out=yout[:, 0:GH],
                            out_offset=bass.IndirectOffsetOnAxis(
                                ap=sidx_t[:, col:col + 1], axis=0
                            ),
                            in_=yst[:],
                            in_offset=None,
                            element_offset=dr * GH,
                        )
                yprev, h16prev = ytile, h16t

        for rep in range(repeat):
            if phases in ("all", "attn"):
                with (
                    tc.tile_pool(name="wt", bufs=2) as wpool,
                    tc.tile_pool(name="ao", bufs=1) as aopool,
                    tc.tile_pool(name="bh", bufs=2) as bhpool,
                    tc.tile_pool(name="psB", bufs=2, space="PSUM") as psB,
                ):
                    attn_phase(wpool, aopool, bhpool, psB)
            if phases in ("all", "gru"):
                with (
                    tc.tile_pool(name="gx", bufs=2) as gxpool,
                    tc.tile_pool(name="rec", bufs=3) as recpool,
                    tc.tile_pool(name="psR", bufs=2, space="PSUM") as psR,
                ):
                    gru_phase(gxpool, recpool, psR)

    nc.compile()
    return nc


_NC_CACHE = {}


def _get_nc(repeat: int = 1):
    if repeat not in _NC_CACHE:
        _NC_CACHE[repeat] = _build(repeat)
    return _NC_CACHE[repeat]


def _host_inputs(inputs, core):
    import ml_dtypes
    f8 = ml_dtypes.float8_e4m3

    bs = slice(core * BC, (core + 1) * BC)
    seg = np.asarray(inputs["seg_feats"][bs])
    seglen = np.asarray(inputs["seglen"][bs]).astype(np.int64)

    m = {
        "xT": np.ascontiguousarray(
            seg.transpose(2, 0, 1).reshape(D, NTOK), dtype=np.float32
        )
    }
    for l in range(NL):
        for nm_in, nm_out in (("Wq", "WqT"), ("Wk", "WkT"), ("Wv", "WvT")):
            m[f"{nm_out}{l}"] = np.ascontiguousarray(
                np.asarray(inputs[nm_in][l]).T, dtype=np.float32
            )
        m[f"WoT{l}"] = np.ascontiguousarray(
            np.asarray(inputs["Wo"][l]).T).astype(ml_dtypes.bfloat16)
    m["WihFT"] = np.ascontiguousarray(np.asarray(inputs["W_ih_f"]).T, np.float32)
    m["WihBT"] = np.ascontiguousarray(np.asarray(inputs["W_ih_b"]).T, np.float32)
    # biases are all zero in this model; the kernel skips them entirely
    for l in range(NL):
        for w in "qkvo":
            assert not np.any(np.asarray(inputs[f"b{w}"][l])), \
                "nonzero attention biases unsupported"
    for nm in ("b_ih_f", "b_ih_b", "b_hh_f", "b_hh_b"):
        assert not np.any(np.asarray(inputs[nm])), "nonzero GRU biases unsupported"
    wtype = f8 if W8 else np.float16
    m["WhhFT"] = np.ascontiguousarray(
        np.asarray(inputs["W_hh_f"]).T * WHH_SCALE).astype(wtype)
    m["WhhBT"] = np.ascontiguousarray(
        np.asarray(inputs["W_hh_b"]).T * WHH_SCALE).astype(wtype)

    # band mask: two 128x128 diagonal blocks + two 3-wide corner blocks
    band = np.zeros((128, 264), np.float32)
    p = np.arange(128)
    for c in range(2):
        band[:, c * 128:(c + 1) * 128] = (
            np.abs(p[:, None] - p[None, :]) <= ATTN_WIDTH
        )
    for j in range(3):
        for pp in range(125, 128):           # corner A: k=pp, q=128+j
            if abs(pp - 128 - j) <= ATTN_WIDTH:
                band[pp, 256 + j] = 1.0
        for pp in range(0, 3):               # corner B: k=128+pp, q=125+j
            if abs(128 + pp - 125 - j) <= ATTN_WIDTH:
                band[pp, 259 + j] = 1.0
    m["band"] = band.astype(ml_dtypes.bfloat16)
    m["ones"] = np.ones((128, 128), ml_dtypes.bfloat16)
    m["iden"] = np.eye(128, dtype=np.float32)

    gxidx = np.zeros((128, NCHUNK * 2), np.int32)
    for ck in range(NCHUNK):
        for hf2 in range(2):
            col = ck * 2 + hf2
            for bl in range(4):
                b = hf2 * 4 + bl
                L = int(seglen[b])
                for jl in range(CH):
                    j = ck * CH + jl
                    src_t = min(max(L - 1 - j, 0), T - 1)
                    gxidx[bl * CH + jl, col] = b * T + src_t
    m["gxidx"] = gxidx

    # scatter rows: partition p = (jl, b) of the transposed y block.
    # Values are 2*row: the kernel's scatter AP has coef=GH (=HID/2), so
    # doubling here makes the index land on row*HID.
    sidx = np.full((128, NCHUNK * 4), 2 * YR, np.int32)
    for ck in range(NCHUNK):
        for dr in range(2):
            for jh in range(2):
                col = ck * 4 + dr * 2 + jh
                for jl in range(16):
                    j = ck * CH + jh * 16 + jl
                    for b in range(BC):
                        L = int(seglen[b])
                        if j < L:
                            t = j if dr == 0 else L - 1 - j
                            sidx[jl * 8 + b, col] = 2 * (b * T + t)
    m["sidx"] = sidx
    return m


def kernel(**inputs) -> np.ndarray:
    repeat = int(os.environ.get("KERNEL_REPEAT", "1"))
    nc = _get_nc(repeat)
    in_maps = [_host_inputs(inputs, c) for c in range(NCORES)]
    res = run_bass_kernel_spmd(nc, in_maps, core_ids=list(range(NCORES)))
    out = np.stack([
        res.results[c]["yout"][0:YR].reshape(BC, T, HID) for c in range(NCORES)
    ])
    return np.ascontiguousarray(
        out.reshape(B, T, HID), dtype=np.float32
    )
